# revision 13
# baseline (speedup 1.0000x reference)
"""AAConv2d (attention-augmented conv) Trainium2 kernel.

Data-parallel over batch: 8 images -> 8 NeuronCores, no collectives.
Per core: qkv projection, 8-head attention with relative-position logits
folded into the QK matmul as extra contraction rows, softmax (no max-sub;
logits are small), PV, out-projection, and a 3x3 conv via 9 shifted
matmuls on a zero-padded input. All matmul operands bf16, PSUM f32.

Layout notes (per head n, base = 0 for even n / 64 for odd n to match the
partition placement of the projection PSUM output):
  Qp[n] (128, 1024): rows base..base+63 = Q_n, the other 64 rows hold
      [Aw (32); Ah (32)] - the gathered relative-logit tables.
  Kp[n] (128, 1024): rows base.. = K_n, other 64 rows = [Ew; Eh] consts.
  logits^T tile (t, s) = Kp-chunk^T @ Qp, includes QK + w_rel + h_rel.
Softmax denominators: per 4-head group, ones^T @ exp matmuls stack the
sums at partition rows {0,32,64,96} of one PSUM tile so a single
reciprocal covers 4 heads (DVE reciprocal cost is per-free-element).
"""
import numpy as np
import ml_dtypes

import concourse.bass as bass
import concourse.tile as tile
from concourse import bacc, mybir
from concourse.bass_utils import run_bass_kernel_spmd

F32 = mybir.dt.float32
BF16 = mybir.dt.bfloat16
AF = mybir.ActivationFunctionType
ALU = mybir.AluOpType

B, CIN, H, W = 8, 256, 32, 32
L = H * W
DK, DV, NH = 512, 256, 8
DKH, DVH = DK // NH, DV // NH

TRACE = False
TRACE_KW = {}
LAST_RESULT = None


def _bf(a):
    return np.ascontiguousarray(a).astype(ml_dtypes.bfloat16)


def build():
    nc = bacc.Bacc("TRN2", target_bir_lowering=False, debug=False, num_devices=8)

    xpad = nc.dram_tensor("xpad", [256, 1156], BF16, kind="ExternalInput")
    xnat = nc.dram_tensor("xnat", [256, 1024], BF16, kind="ExternalInput")
    wqkt = nc.dram_tensor("wqkt", [256, 1024], BF16, kind="ExternalInput")
    wvt = nc.dram_tensor("wvt", [256, 256], BF16, kind="ExternalInput")
    wconvt = nc.dram_tensor("wconvt", [256, 2304], BF16, kind="ExternalInput")
    woutt = nc.dram_tensor("woutt", [256, 256], BF16, kind="ExternalInput")
    relw = nc.dram_tensor("relw", [128, 126], BF16, kind="ExternalInput")
    econst = nc.dram_tensor("econst", [64, 1024], BF16, kind="ExternalInput")
    out_d = nc.dram_tensor("out", [512, 1024], F32, kind="ExternalOutput")
    tdram = nc.dram_tensor("tdram", [8, 128, 1024], BF16)  # rel-table scratch

    with tile.TileContext(nc) as tc:
        with (
            tc.tile_pool(name="const", bufs=1) as cpool,
            tc.tile_pool(name="qp", bufs=1) as qpool,
            tc.tile_pool(name="kp", bufs=1) as kpool,
            tc.tile_pool(name="vto", bufs=1) as vpool,
            tc.tile_pool(name="attn", bufs=1) as apool,
            tc.tile_pool(name="wo", bufs=1) as wopool,
            tc.tile_pool(name="expt", bufs=10) as epool,
            tc.tile_pool(name="stage", bufs=2) as stpool,
            tc.tile_pool(name="scratch", bufs=2) as scpool,
            tc.tile_pool(name="outsb", bufs=4) as opool,
            tc.tile_pool(name="small_sb", bufs=4) as sspool,
            tc.tile_pool(name="rec_sb", bufs=2) as rpool,
            tc.tile_pool(name="bigps", bufs=2, space="PSUM") as bigps,
            tc.tile_pool(name="convps", bufs=1, space="PSUM") as cvps,
            tc.tile_pool(name="attbc", bufs=2, space="PSUM") as abps,
        ):
            # ---- load inputs: latency-critical ones on HWDGE (sync),
            # ---- bulky late-use weights on SWDGE (gpsimd) in parallel ----
            xp_sb = [cpool.tile([128, 1156], BF16, tag=f"xp{c}", name=f"xp{c}") for c in range(2)]
            xn_sb = [cpool.tile([128, 1024], BF16, tag=f"xn{c}", name=f"xn{c}") for c in range(2)]
            wqk_sb = [cpool.tile([128, 1024], BF16, tag=f"wqk{c}", name=f"wqk{c}") for c in range(2)]
            wv_sb = [cpool.tile([128, 256], BF16, tag=f"wv{c}", name=f"wv{c}") for c in range(2)]
            wcv_sb = [cpool.tile([128, 2304], BF16, tag=f"wcv{c}", name=f"wcv{c}") for c in range(2)]
            rel_sb = cpool.tile([128, 126], BF16, tag="rel")
            nc.sync.dma_start(wqk_sb[0][:], wqkt.ap()[0:128, :])
            nc.scalar.dma_start(wqk_sb[1][:], wqkt.ap()[128:256, :])
            nc.sync.dma_start(xp_sb[0][:], xpad.ap()[0:128, :])
            nc.scalar.dma_start(xp_sb[1][:], xpad.ap()[128:256, :])
            nc.scalar.dma_start(rel_sb[:], relw.ap())
            nc.sync.dma_start(xn_sb[0][:], xnat.ap()[0:128, :])
            nc.scalar.dma_start(xn_sb[1][:], xnat.ap()[128:256, :])
            for c in range(2):
                nc.sync.dma_start(wv_sb[c][:], wvt.ap()[128 * c:128 * c + 128, :])
                nc.sync.dma_start(wcv_sb[c][:], wconvt.ap()[128 * c:128 * c + 128, :])
            wo_sb = []
            for n in range(8):
                t = wopool.tile([32, 256], BF16, name=f"wo{n}")
                nc.sync.dma_start(t[:], woutt.ap()[32 * n:32 * n + 32, :])
                wo_sb.append(t)
            ones128 = cpool.tile([128, 32], BF16, tag="ones")
            nc.vector.memset(ones128[:], 1.0)
            onescol = cpool.tile([128, 1], BF16, tag="onescol")
            nc.vector.memset(onescol[:], 1.0)

            # interior view of padded x: (128, h 32, w 32), h-stride 34
            def xin(c):
                return xp_sb[c][:].rearrange("p (h w) -> p h w", h=34)[:, 1:33, 1:33]

            qp = [qpool.tile([128, 1024], BF16, name=f"qp{i}") for i in range(8)]
            kp = [kpool.tile([128, 1024], BF16, name=f"kp{i}") for i in range(8)]

            def proj_chunk(m, dest):
                ps = bigps.tile([128, 1024], F32, tag="big", name="projps")
                for c in range(2):
                    for s in range(2):
                        nc.tensor.matmul(
                            ps[:, 512 * s:512 * s + 512],
                            wqk_sb[c][:, 128 * m:128 * m + 128],
                            xin(c)[:, 16 * s:16 * s + 16, :],
                            start=(c == 0), stop=(c == 1),
                        )
                h0 = 2 * (m % 4)
                nc.vector.tensor_copy(dest[h0][0:64, :], ps[0:64, :])
                nc.vector.tensor_copy(dest[h0 + 1][64:128, :], ps[64:128, :])

            def head_tables(n):
                """rel-table matmuls -> stage -> DRAM -> gathers -> sigma copy."""
                base = 0 if n % 2 == 0 else 64
                aw_b = 64 - base
                qn = qp[n][base:base + 64, :]
                qsig = qn.rearrange("p (a b) -> p a b", a=32).transpose([0, 2, 1])
                tps = bigps.tile([128, 1024], F32, tag="big", name="tps")
                for c in range(2):
                    nc.tensor.matmul(
                        tps[0:63, 512 * c:512 * c + 512],
                        rel_sb[base:base + 64, 0:63],
                        qsig[:, 16 * c:16 * c + 16, :],
                        start=True, stop=True,
                    )
                for c in range(2):
                    nc.tensor.matmul(
                        tps[64:127, 512 * c:512 * c + 512],
                        rel_sb[base:base + 64, 63:126],
                        qn[:, 512 * c:512 * c + 512],
                        start=True, stop=True,
                    )
                stg = stpool.tile([128, 1024], BF16, name="stg")
                nc.scalar.copy(stg[:], tps[:])
                nc.sync.dma_start(tdram.ap()[n, :, :], stg[:])
                nc.sync.dma_start(
                    qp[n][aw_b:aw_b + 32, :].rearrange("p (a b) -> p a b", a=32),
                    bass.AP(tdram, n * 131072 + 31 * 1024,
                            [[1024, 32], [-992, 32], [1, 32]]),
                )
                sc = scpool.tile([128, 1024], BF16, name="scr")
                ah_b = aw_b + 32
                nc.sync.dma_start(
                    sc[ah_b:ah_b + 32, :].rearrange("p (a b) -> p a b", a=32),
                    bass.AP(tdram, n * 131072 + 65536 + 31 * 1024,
                            [[1024, 32], [-992, 32], [1, 32]]),
                )
                dst3 = qp[n][ah_b:ah_b + 32, :].rearrange("p (a b) -> p a b", a=32)
                src3 = (sc[ah_b:ah_b + 32, :].rearrange("p (a b) -> p a b", a=32)
                        .transpose([0, 2, 1]))
                nc.vector.tensor_copy(dst3[:, 0:16, :], src3[:, 0:16, :])
                nc.gpsimd.tensor_copy(dst3[:, 16:32, :], src3[:, 16:32, :])
                nc.scalar.dma_start(kp[n][aw_b:aw_b + 64, :], econst.ap())

            # q projection + rel tables interleaved (gathers start early)
            for m in range(4):
                proj_chunk(m, qp)
                head_tables(2 * m)
                head_tables(2 * m + 1)
            # k projection
            for m in range(4, 8):
                proj_chunk(m, kp)

            # ---- vT projection: vT_all (t, head-major c) + ones col ----
            vto = []
            for j in range(8):
                ps = bigps.tile([128, 256], F32, tag="big", name="vps")
                for c in range(2):
                    nc.tensor.matmul(
                        ps[:], xn_sb[c][:, 128 * j:128 * j + 128], wv_sb[c][:],
                        start=(c == 0), stop=(c == 1),
                    )
                t = vpool.tile([128, 264], BF16, name=f"vto{j}")
                nc.vector.memset(t[:], 1.0)  # cols 33n+32 stay 1.0
                nc.vector.tensor_copy(
                    t[:].rearrange("p (n c) -> p n c", n=8)[:, :, 0:32],
                    ps[:].rearrange("p (n c) -> p n c", n=8),
                )
                vto.append(t)

            def conv_group(o):
                ps = cvps.tile([128, 1024], F32, tag="cv", name="cps")
                for tap in range(9):
                    dy, dx = tap // 3, tap % 3
                    for c in range(2):
                        for hh in range(2):
                            rhs = (xp_sb[c][:]
                                   .rearrange("p (h w) -> p h w", h=34)
                                   [:, dy + 16 * hh:dy + 16 * hh + 16, dx:dx + 32])
                            nc.tensor.matmul(
                                ps[:, 512 * hh:512 * hh + 512],
                                wcv_sb[c][:, 256 * tap + 128 * o:256 * tap + 128 * o + 128],
                                rhs,
                                start=(tap == 0 and c == 0),
                                stop=(tap == 8 and c == 1),
                                skip_group_check=True,
                            )
                osb = opool.tile([128, 1024], F32, name="osb2")
                nc.vector.tensor_copy(osb[:], ps[:])
                nc.sync.dma_start(out_d.ap()[128 * o:128 * o + 128, :], osb[:])

            # conv o=0 fills the PE while rel-table gathers land
            conv_group(0)

            # ---- attention: compute all heads (PV fused with denom row),
            # ---- then normalize (keeps PE off the reciprocal's tail) ----
            att_sb = [apool.tile([32, 1024], BF16, name=f"att{i}") for i in range(8)]
            araw = {}

            def group_recip(grp):
                recf = rpool.tile([97, 1024], F32, tag="recf", name="recf")
                rec = rpool.tile([97, 1024], BF16, name="rec")
                for c in range(2):
                    sg = sspool.tile([97, 512], F32, tag=f"sg{grp}{c}",
                                     name=f"sg{grp}{c}", bufs=1)
                    for g in range(4):
                        nc.sync.dma_start(sg[32 * g:32 * g + 1, :],
                                          araw[(4 * grp + g, c)][32:33, :])
                    nc.vector.reciprocal_approx_fast(
                        out=recf[0:97, 512 * c:512 * c + 512], in_=sg[:])
                    nc.vector.tensor_copy(rec[0:97, 512 * c:512 * c + 512],
                                          recf[0:97, 512 * c:512 * c + 512])
                return rec

            def group_norm(grp, rec):
                for g in range(4):
                    n = 4 * grp + g
                    for c in range(2):
                        bc = abps.tile([32, 512], F32, tag="ab", name="bc")
                        nc.tensor.matmul(
                            bc[:], ones128[32 * g:32 * g + 1, 0:32],
                            rec[32 * g:32 * g + 1, 512 * c:512 * c + 512],
                            start=True, stop=True,
                            tile_position=(32 * g, 0),
                        )
                        bcs = sspool.tile([32, 512], F32, tag="bcs", name="bcs")
                        nc.scalar.copy(bcs[:], bc[:])
                        nc.vector.tensor_tensor(
                            att_sb[n][:, 512 * c:512 * c + 512],
                            araw[(n, c)][0:32, :], bcs[:], op=ALU.mult,
                        )
            for n in range(8):
                expt = []
                for j in range(8):
                    lt = bigps.tile([128, 1024], F32, tag="big", name="lt")
                    for c in range(2):
                        nc.tensor.matmul(
                            lt[:, 512 * c:512 * c + 512],
                            kp[n][:, 128 * j:128 * j + 128],
                            qp[n][:, 512 * c:512 * c + 512],
                            start=True, stop=True,
                        )
                    et = epool.tile([128, 1024], BF16, name="et")
                    nc.scalar.activation(et[:], lt[:], AF.Exp)
                    expt.append(et)
                aps2 = [abps.tile([33, 512], F32, tag="ab", name=f"aps{c}")
                        for c in range(2)]
                for j in range(8):
                    for c in range(2):
                        nc.tensor.matmul(
                            aps2[c][:],
                            vto[j][:, 33 * n:33 * n + 33],
                            expt[j][:, 512 * c:512 * c + 512],
                            start=(j == 0), stop=(j == 7),
                            skip_group_check=True,
                        )
                for c in range(2):
                    ar = sspool.tile([33, 512], F32, tag="araw",
                                     name="araw", bufs=16)
                    nc.vector.tensor_copy(ar[:], aps2[c][:])
                    araw[(n, c)] = ar
                if n == 3:
                    conv_group(1)
                    rec0 = group_recip(0)
                if n == 4:
                    group_norm(0, rec0)


            group_norm(1, group_recip(1))

            # ---- attn out-projection -> out rows 256..511 ----
            for o in range(2):
                ps = bigps.tile([128, 1024], F32, tag="big", name="pout")
                for n in range(8):
                    for c in range(2):
                        nc.tensor.matmul(
                            ps[:, 512 * c:512 * c + 512],
                            wo_sb[n][:, 128 * o:128 * o + 128],
                            att_sb[n][:, 512 * c:512 * c + 512],
                            start=(n == 0), stop=(n == 7),
                            skip_group_check=True,
                        )
                osb = opool.tile([128, 1024], F32, name="osb")
                nc.vector.tensor_copy(osb[:], ps[:])
                nc.sync.dma_start(out_d.ap()[256 + 128 * o:384 + 128 * o, :], osb[:])


    nc.compile()
    return nc


_NC_CACHE = None


def kernel(x, w_qkv, w_conv, w_out, key_rel_h, key_rel_w):
    global _NC_CACHE, LAST_RESULT
    x = np.asarray(x, np.float32)
    w_qkv = np.asarray(w_qkv, np.float32)
    w_conv = np.asarray(w_conv, np.float32)
    w_out = np.asarray(w_out, np.float32)
    key_rel_h = np.asarray(key_rel_h, np.float32)
    key_rel_w = np.asarray(key_rel_w, np.float32)

    wq = w_qkv.copy()
    wq[:DK] *= DKH ** -0.5
    wqkt = _bf(wq[:1024].T)                      # (256, 1024)
    wvt = _bf(wq[1024:].T)                       # (256, 256)
    wconvt = _bf(w_conv.transpose(1, 2, 3, 0).reshape(256, 9 * 256))
    woutt = _bf(w_out.T)
    rel2 = np.concatenate([key_rel_w, key_rel_h], axis=1)  # (64, 126)
    relw = _bf(np.concatenate([rel2, rel2], axis=0))       # (128, 126)
    t = np.arange(L)
    ew = (t[None, :] // 32 == np.arange(32)[:, None]).astype(np.float32)
    eh = (t[None, :] % 32 == np.arange(32)[:, None]).astype(np.float32)
    econst = _bf(np.concatenate([ew, eh], axis=0))         # (64, 1024)

    shared = dict(wqkt=wqkt, wvt=wvt, wconvt=wconvt, woutt=woutt,
                  relw=relw, econst=econst)
    in_maps = []
    for b in range(B):
        xp = np.zeros((256, 34, 34), np.float32)
        xp[:, 1:33, 1:33] = x[b]
        in_maps.append(dict(shared, xpad=_bf(xp.reshape(256, 1156)),
                            xnat=_bf(x[b].reshape(256, 1024))))

    if _NC_CACHE is None:
        _NC_CACHE = build()
    res = run_bass_kernel_spmd(_NC_CACHE, in_maps, core_ids=list(range(8)),
                               trace=TRACE, **TRACE_KW)
    LAST_RESULT = res
    out = np.stack([res.results[i]["out"] for i in range(B)])
    return out.reshape(B, 512, H, W).astype(np.float32)


# revision 14
# speedup vs baseline: 1.1850x; 1.1850x over previous
"""AAConv2d (attention-augmented conv) Trainium2 kernel.

Data-parallel over batch: 8 images -> 8 NeuronCores, no collectives.
Per core: qkv projection, 8-head attention with relative-position logits
folded into the QK matmul as extra contraction rows, softmax (no max-sub;
logits are small), PV, out-projection, and a 3x3 conv via 9 shifted
matmuls on a zero-padded input. All matmul operands bf16, PSUM f32.

Layout notes (per head n, base = 0 for even n / 64 for odd n to match the
partition placement of the projection PSUM output):
  Qp[n] (128, 1024): rows base..base+63 = Q_n, the other 64 rows hold
      [Aw (32); Ah (32)] - the gathered relative-logit tables.
  Kp[n] (128, 1024): rows base.. = K_n, other 64 rows = [Ew; Eh] consts.
  logits^T tile (t, s) = Kp-chunk^T @ Qp, includes QK + w_rel + h_rel.
Softmax denominators: per 4-head group, ones^T @ exp matmuls stack the
sums at partition rows {0,32,64,96} of one PSUM tile so a single
reciprocal covers 4 heads (DVE reciprocal cost is per-free-element).
"""
import numpy as np
import ml_dtypes

import concourse.bass as bass
import concourse.tile as tile
from concourse import bacc, mybir
from concourse.bass_utils import run_bass_kernel_spmd

F32 = mybir.dt.float32
BF16 = mybir.dt.bfloat16
AF = mybir.ActivationFunctionType
ALU = mybir.AluOpType

B, CIN, H, W = 8, 256, 32, 32
L = H * W
DK, DV, NH = 512, 256, 8
DKH, DVH = DK // NH, DV // NH

TRACE = False
TRACE_KW = {}
LAST_RESULT = None


def _bf(a):
    return np.ascontiguousarray(a).astype(ml_dtypes.bfloat16)


def build():
    nc = bacc.Bacc("TRN2", target_bir_lowering=False, debug=False, num_devices=8)

    xpad = nc.dram_tensor("xpad", [256, 1156], BF16, kind="ExternalInput")
    xnat = nc.dram_tensor("xnat", [256, 1024], BF16, kind="ExternalInput")
    wqkt = nc.dram_tensor("wqkt", [256, 1024], BF16, kind="ExternalInput")
    wvt = nc.dram_tensor("wvt", [256, 256], BF16, kind="ExternalInput")
    wconvt = nc.dram_tensor("wconvt", [256, 2304], BF16, kind="ExternalInput")
    woutt = nc.dram_tensor("woutt", [256, 256], BF16, kind="ExternalInput")
    relw = nc.dram_tensor("relw", [128, 126], BF16, kind="ExternalInput")
    econst = nc.dram_tensor("econst", [64, 1024], BF16, kind="ExternalInput")
    out_d = nc.dram_tensor("out", [512, 1024], F32, kind="ExternalOutput")
    tdram = nc.dram_tensor("tdram", [8, 128, 1024], BF16)  # rel-table scratch

    with tile.TileContext(nc) as tc:
        with (
            tc.tile_pool(name="const", bufs=1) as cpool,
            tc.tile_pool(name="qp", bufs=1) as qpool,
            tc.tile_pool(name="kp", bufs=1) as kpool,
            tc.tile_pool(name="vto", bufs=1) as vpool,
            tc.tile_pool(name="attn", bufs=1) as apool,
            tc.tile_pool(name="wo", bufs=1) as wopool,
            tc.tile_pool(name="expt", bufs=10) as epool,
            tc.tile_pool(name="stage", bufs=2) as stpool,
            tc.tile_pool(name="scratch", bufs=2) as scpool,
            tc.tile_pool(name="outsb", bufs=4) as opool,
            tc.tile_pool(name="small_sb", bufs=4) as sspool,
            tc.tile_pool(name="rec_sb", bufs=2) as rpool,
            tc.tile_pool(name="bigps", bufs=2, space="PSUM") as bigps,
            tc.tile_pool(name="convps", bufs=1, space="PSUM") as cvps,
            tc.tile_pool(name="attbc", bufs=2, space="PSUM") as abps,
        ):
            # ---- load inputs: latency-critical ones on HWDGE (sync),
            # ---- bulky late-use weights on SWDGE (gpsimd) in parallel ----
            xp_sb = [cpool.tile([128, 1156], BF16, tag=f"xp{c}", name=f"xp{c}") for c in range(2)]
            xn_sb = [cpool.tile([128, 1024], BF16, tag=f"xn{c}", name=f"xn{c}") for c in range(2)]
            wqk_sb = [cpool.tile([128, 1024], BF16, tag=f"wqk{c}", name=f"wqk{c}") for c in range(2)]
            wv_sb = [cpool.tile([128, 256], BF16, tag=f"wv{c}", name=f"wv{c}") for c in range(2)]
            wcv_sb = [cpool.tile([128, 2304], BF16, tag=f"wcv{c}", name=f"wcv{c}") for c in range(2)]
            rel_sb = cpool.tile([128, 126], BF16, tag="rel")
            nc.sync.dma_start(wqk_sb[0][:], wqkt.ap()[0:128, :])
            nc.scalar.dma_start(wqk_sb[1][:], wqkt.ap()[128:256, :])
            nc.sync.dma_start(xp_sb[0][:], xpad.ap()[0:128, :])
            nc.scalar.dma_start(xp_sb[1][:], xpad.ap()[128:256, :])
            nc.scalar.dma_start(rel_sb[:], relw.ap())
            nc.sync.dma_start(xn_sb[0][:], xnat.ap()[0:128, :])
            nc.scalar.dma_start(xn_sb[1][:], xnat.ap()[128:256, :])
            for c in range(2):
                nc.scalar.dma_start(wv_sb[c][:], wvt.ap()[128 * c:128 * c + 128, :])
                nc.scalar.dma_start(wcv_sb[c][:], wconvt.ap()[128 * c:128 * c + 128, :])
            wo_sb = []
            for n in range(8):
                t = wopool.tile([32, 256], BF16, name=f"wo{n}")
                nc.scalar.dma_start(t[:], woutt.ap()[32 * n:32 * n + 32, :])
                wo_sb.append(t)
            ones128 = cpool.tile([128, 32], BF16, tag="ones")
            nc.vector.memset(ones128[:], 1.0)
            onescol = cpool.tile([128, 1], BF16, tag="onescol")
            nc.vector.memset(onescol[:], 1.0)

            # interior view of padded x: (128, h 32, w 32), h-stride 34
            def xin(c):
                return xp_sb[c][:].rearrange("p (h w) -> p h w", h=34)[:, 1:33, 1:33]

            qp = [qpool.tile([128, 1024], BF16, name=f"qp{i}") for i in range(8)]
            kp = [kpool.tile([128, 1024], BF16, name=f"kp{i}") for i in range(8)]

            def proj_chunk(m, dest):
                ps = bigps.tile([128, 1024], F32, tag="big", name="projps")
                for c in range(2):
                    for s in range(2):
                        nc.tensor.matmul(
                            ps[:, 512 * s:512 * s + 512],
                            wqk_sb[c][:, 128 * m:128 * m + 128],
                            xin(c)[:, 16 * s:16 * s + 16, :],
                            start=(c == 0), stop=(c == 1),
                        )
                h0 = 2 * (m % 4)
                nc.vector.tensor_copy(dest[h0][0:64, :], ps[0:64, :])
                nc.vector.tensor_copy(dest[h0 + 1][64:128, :], ps[64:128, :])

            def head_tables(n):
                """rel-table matmuls -> stage -> DRAM -> gathers -> sigma copy."""
                base = 0 if n % 2 == 0 else 64
                aw_b = 64 - base
                qn = qp[n][base:base + 64, :]
                qsig = qn.rearrange("p (a b) -> p a b", a=32).transpose([0, 2, 1])
                tps = bigps.tile([128, 1024], F32, tag="big", name="tps")
                for c in range(2):
                    nc.tensor.matmul(
                        tps[0:63, 512 * c:512 * c + 512],
                        rel_sb[base:base + 64, 0:63],
                        qsig[:, 16 * c:16 * c + 16, :],
                        start=True, stop=True,
                    )
                for c in range(2):
                    nc.tensor.matmul(
                        tps[64:127, 512 * c:512 * c + 512],
                        rel_sb[base:base + 64, 63:126],
                        qn[:, 512 * c:512 * c + 512],
                        start=True, stop=True,
                    )
                stg = stpool.tile([128, 1024], BF16, name="stg")
                nc.scalar.copy(stg[:], tps[:])
                nc.sync.dma_start(tdram.ap()[n, :, :], stg[:])
                nc.sync.dma_start(
                    qp[n][aw_b:aw_b + 32, :].rearrange("p (a b) -> p a b", a=32),
                    bass.AP(tdram, n * 131072 + 31 * 1024,
                            [[1024, 32], [-992, 32], [1, 32]]),
                )
                sc = scpool.tile([128, 1024], BF16, name="scr")
                ah_b = aw_b + 32
                nc.sync.dma_start(
                    sc[ah_b:ah_b + 32, :].rearrange("p (a b) -> p a b", a=32),
                    bass.AP(tdram, n * 131072 + 65536 + 31 * 1024,
                            [[1024, 32], [-992, 32], [1, 32]]),
                )
                dst3 = qp[n][ah_b:ah_b + 32, :].rearrange("p (a b) -> p a b", a=32)
                src3 = (sc[ah_b:ah_b + 32, :].rearrange("p (a b) -> p a b", a=32)
                        .transpose([0, 2, 1]))
                nc.vector.tensor_copy(dst3[:, 0:16, :], src3[:, 0:16, :])
                nc.gpsimd.tensor_copy(dst3[:, 16:32, :], src3[:, 16:32, :])
                nc.scalar.dma_start(kp[n][aw_b:aw_b + 64, :], econst.ap())

            # q projection + rel tables interleaved (gathers start early)
            for m in range(4):
                proj_chunk(m, qp)
                head_tables(2 * m)
                head_tables(2 * m + 1)
            # k projection
            for m in range(4, 8):
                proj_chunk(m, kp)

            # ---- vT projection: vT_all (t, head-major c) + ones col ----
            vto = []
            for j in range(8):
                ps = bigps.tile([128, 256], F32, tag="big", name="vps")
                for c in range(2):
                    nc.tensor.matmul(
                        ps[:], xn_sb[c][:, 128 * j:128 * j + 128], wv_sb[c][:],
                        start=(c == 0), stop=(c == 1),
                    )
                t = vpool.tile([128, 264], BF16, name=f"vto{j}")
                nc.vector.memset(t[:], 1.0)  # cols 33n+32 stay 1.0
                nc.vector.tensor_copy(
                    t[:].rearrange("p (n c) -> p n c", n=8)[:, :, 0:32],
                    ps[:].rearrange("p (n c) -> p n c", n=8),
                )
                vto.append(t)

            def conv_group(o):
                ps = cvps.tile([128, 1024], F32, tag="cv", name="cps")
                for tap in range(9):
                    dy, dx = tap // 3, tap % 3
                    for c in range(2):
                        for hh in range(2):
                            rhs = (xp_sb[c][:]
                                   .rearrange("p (h w) -> p h w", h=34)
                                   [:, dy + 16 * hh:dy + 16 * hh + 16, dx:dx + 32])
                            nc.tensor.matmul(
                                ps[:, 512 * hh:512 * hh + 512],
                                wcv_sb[c][:, 256 * tap + 128 * o:256 * tap + 128 * o + 128],
                                rhs,
                                start=(tap == 0 and c == 0),
                                stop=(tap == 8 and c == 1),
                                skip_group_check=True,
                            )
                osb = opool.tile([128, 1024], F32, name="osb2")
                nc.vector.tensor_copy(osb[:], ps[:])
                nc.sync.dma_start(out_d.ap()[128 * o:128 * o + 128, :], osb[:])

            # conv o=0 fills the PE while rel-table gathers land
            conv_group(0)

            # ---- attention: compute all heads (PV fused with denom row),
            # ---- then normalize (keeps PE off the reciprocal's tail) ----
            att_sb = [apool.tile([32, 1024], BF16, name=f"att{i}") for i in range(8)]
            araw = {}

            def group_recip(grp):
                recf = rpool.tile([97, 1024], F32, tag="recf", name="recf")
                rec = rpool.tile([97, 1024], BF16, name="rec")
                for c in range(2):
                    sg = sspool.tile([97, 512], F32, tag=f"sg{grp}{c}",
                                     name=f"sg{grp}{c}", bufs=1)
                    for g in range(4):
                        nc.sync.dma_start(sg[32 * g:32 * g + 1, :],
                                          araw[(4 * grp + g, c)][32:33, :])
                    nc.vector.reciprocal_approx_fast(
                        out=recf[0:97, 512 * c:512 * c + 512], in_=sg[:])
                    nc.vector.tensor_copy(rec[0:97, 512 * c:512 * c + 512],
                                          recf[0:97, 512 * c:512 * c + 512])
                return rec

            def group_norm(grp, rec):
                for g in range(4):
                    n = 4 * grp + g
                    for c in range(2):
                        bc = abps.tile([32, 512], F32, tag="ab", name="bc")
                        nc.tensor.matmul(
                            bc[:], ones128[32 * g:32 * g + 1, 0:32],
                            rec[32 * g:32 * g + 1, 512 * c:512 * c + 512],
                            start=True, stop=True,
                            tile_position=(32 * g, 0),
                        )
                        bcs = sspool.tile([32, 512], F32, tag="bcs", name="bcs")
                        nc.scalar.copy(bcs[:], bc[:])
                        nc.vector.tensor_tensor(
                            att_sb[n][:, 512 * c:512 * c + 512],
                            araw[(n, c)][0:32, :], bcs[:], op=ALU.mult,
                        )
            for n in range(8):
                expt = []
                for j in range(8):
                    lt = bigps.tile([128, 1024], F32, tag="big", name="lt")
                    for c in range(2):
                        nc.tensor.matmul(
                            lt[:, 512 * c:512 * c + 512],
                            kp[n][:, 128 * j:128 * j + 128],
                            qp[n][:, 512 * c:512 * c + 512],
                            start=True, stop=True,
                        )
                    et = epool.tile([128, 1024], BF16, name="et")
                    nc.scalar.activation(et[:], lt[:], AF.Exp)
                    expt.append(et)
                aps2 = [abps.tile([33, 512], F32, tag="ab", name=f"aps{c}")
                        for c in range(2)]
                for j in range(8):
                    for c in range(2):
                        nc.tensor.matmul(
                            aps2[c][:],
                            vto[j][:, 33 * n:33 * n + 33],
                            expt[j][:, 512 * c:512 * c + 512],
                            start=(j == 0), stop=(j == 7),
                            skip_group_check=True,
                        )
                for c in range(2):
                    ar = sspool.tile([33, 512], F32, tag="araw",
                                     name="araw", bufs=16)
                    nc.vector.tensor_copy(ar[:], aps2[c][:])
                    araw[(n, c)] = ar
                if n == 3:
                    conv_group(1)
                    rec0 = group_recip(0)
                if n == 4:
                    group_norm(0, rec0)


            group_norm(1, group_recip(1))

            # ---- attn out-projection -> out rows 256..511 ----
            for o in range(2):
                ps = bigps.tile([128, 1024], F32, tag="big", name="pout")
                for n in range(8):
                    for c in range(2):
                        nc.tensor.matmul(
                            ps[:, 512 * c:512 * c + 512],
                            wo_sb[n][:, 128 * o:128 * o + 128],
                            att_sb[n][:, 512 * c:512 * c + 512],
                            start=(n == 0), stop=(n == 7),
                            skip_group_check=True,
                        )
                osb = opool.tile([128, 1024], F32, name="osb")
                nc.vector.tensor_copy(osb[:], ps[:])
                nc.sync.dma_start(out_d.ap()[256 + 128 * o:384 + 128 * o, :], osb[:])


    nc.compile()
    return nc


_NC_CACHE = None


def kernel(x, w_qkv, w_conv, w_out, key_rel_h, key_rel_w):
    global _NC_CACHE, LAST_RESULT
    x = np.asarray(x, np.float32)
    w_qkv = np.asarray(w_qkv, np.float32)
    w_conv = np.asarray(w_conv, np.float32)
    w_out = np.asarray(w_out, np.float32)
    key_rel_h = np.asarray(key_rel_h, np.float32)
    key_rel_w = np.asarray(key_rel_w, np.float32)

    wq = w_qkv.copy()
    wq[:DK] *= DKH ** -0.5
    wqkt = _bf(wq[:1024].T)                      # (256, 1024)
    wvt = _bf(wq[1024:].T)                       # (256, 256)
    wconvt = _bf(w_conv.transpose(1, 2, 3, 0).reshape(256, 9 * 256))
    woutt = _bf(w_out.T)
    rel2 = np.concatenate([key_rel_w, key_rel_h], axis=1)  # (64, 126)
    relw = _bf(np.concatenate([rel2, rel2], axis=0))       # (128, 126)
    t = np.arange(L)
    ew = (t[None, :] // 32 == np.arange(32)[:, None]).astype(np.float32)
    eh = (t[None, :] % 32 == np.arange(32)[:, None]).astype(np.float32)
    econst = _bf(np.concatenate([ew, eh], axis=0))         # (64, 1024)

    shared = dict(wqkt=wqkt, wvt=wvt, wconvt=wconvt, woutt=woutt,
                  relw=relw, econst=econst)
    in_maps = []
    for b in range(B):
        xp = np.zeros((256, 34, 34), np.float32)
        xp[:, 1:33, 1:33] = x[b]
        in_maps.append(dict(shared, xpad=_bf(xp.reshape(256, 1156)),
                            xnat=_bf(x[b].reshape(256, 1024))))

    if _NC_CACHE is None:
        _NC_CACHE = build()
    res = run_bass_kernel_spmd(_NC_CACHE, in_maps, core_ids=list(range(8)),
                               trace=TRACE, **TRACE_KW)
    LAST_RESULT = res
    out = np.stack([res.results[i]["out"] for i in range(B)])
    return out.reshape(B, 512, H, W).astype(np.float32)


# revision 15
# speedup vs baseline: 1.2531x; 1.0574x over previous
"""AAConv2d (attention-augmented conv) Trainium2 kernel.

Data-parallel over batch: 8 images -> 8 NeuronCores, no collectives.
Per core: qkv projection, 8-head attention with relative-position logits
folded into the QK matmul as extra contraction rows, softmax (no max-sub;
logits are small), PV, out-projection, and a 3x3 conv via 9 shifted
matmuls on a zero-padded input. All matmul operands bf16, PSUM f32.

Layout notes (per head n, base = 0 for even n / 64 for odd n to match the
partition placement of the projection PSUM output):
  Qp[n] (128, 1024): rows base..base+63 = Q_n, the other 64 rows hold
      [Aw (32); Ah (32)] - the gathered relative-logit tables.
  Kp[n] (128, 1024): rows base.. = K_n, other 64 rows = [Ew; Eh] consts.
  logits^T tile (t, s) = Kp-chunk^T @ Qp, includes QK + w_rel + h_rel.
Softmax denominators: per 4-head group, ones^T @ exp matmuls stack the
sums at partition rows {0,32,64,96} of one PSUM tile so a single
reciprocal covers 4 heads (DVE reciprocal cost is per-free-element).
"""
import numpy as np
import ml_dtypes

import concourse.bass as bass
import concourse.tile as tile
from concourse import bacc, mybir
from concourse.bass_utils import run_bass_kernel_spmd

F32 = mybir.dt.float32
BF16 = mybir.dt.bfloat16
AF = mybir.ActivationFunctionType
ALU = mybir.AluOpType

B, CIN, H, W = 8, 256, 32, 32
L = H * W
DK, DV, NH = 512, 256, 8
DKH, DVH = DK // NH, DV // NH

TRACE = False
TRACE_KW = {}
LAST_RESULT = None


def _bf(a):
    return np.ascontiguousarray(a).astype(ml_dtypes.bfloat16)


def build():
    nc = bacc.Bacc("TRN2", target_bir_lowering=False, debug=False, num_devices=8)

    xpad = nc.dram_tensor("xpad", [256, 1156], BF16, kind="ExternalInput")
    xnat = nc.dram_tensor("xnat", [256, 1024], BF16, kind="ExternalInput")
    wqkt = nc.dram_tensor("wqkt", [256, 1024], BF16, kind="ExternalInput")
    wvt = nc.dram_tensor("wvt", [256, 256], BF16, kind="ExternalInput")
    wconvt = nc.dram_tensor("wconvt", [256, 2304], BF16, kind="ExternalInput")
    woutt = nc.dram_tensor("woutt", [256, 256], BF16, kind="ExternalInput")
    relw = nc.dram_tensor("relw", [128, 126], BF16, kind="ExternalInput")
    econst = nc.dram_tensor("econst", [128, 1024], BF16, kind="ExternalInput")
    out_d = nc.dram_tensor("out", [512, 1024], F32, kind="ExternalOutput")
    tdram = nc.dram_tensor("tdram", [8, 128, 1024], BF16)  # rel-table scratch

    with tile.TileContext(nc) as tc:
        with (
            tc.tile_pool(name="const", bufs=1) as cpool,
            tc.tile_pool(name="qp", bufs=1) as qpool,
            tc.tile_pool(name="kp", bufs=1) as kpool,
            tc.tile_pool(name="vto", bufs=1) as vpool,
            tc.tile_pool(name="attn", bufs=1) as apool,
            tc.tile_pool(name="wo", bufs=1) as wopool,
            tc.tile_pool(name="expt", bufs=12) as epool,
            tc.tile_pool(name="stage", bufs=2) as stpool,
            tc.tile_pool(name="scratch", bufs=2) as scpool,
            tc.tile_pool(name="outsb", bufs=4) as opool,
            tc.tile_pool(name="small_sb", bufs=4) as sspool,
            tc.tile_pool(name="rec_sb", bufs=2) as rpool,
            tc.tile_pool(name="bigps", bufs=2, space="PSUM") as bigps,
            tc.tile_pool(name="convps", bufs=1, space="PSUM") as cvps,
            tc.tile_pool(name="attbc", bufs=2, space="PSUM") as abps,
        ):
            # ---- load inputs: latency-critical ones on HWDGE (sync),
            # ---- bulky late-use weights on SWDGE (gpsimd) in parallel ----
            xp_sb = [cpool.tile([128, 1156], BF16, tag=f"xp{c}", name=f"xp{c}") for c in range(2)]
            xn_sb = [cpool.tile([128, 1024], BF16, tag=f"xn{c}", name=f"xn{c}") for c in range(2)]
            wqk_sb = [cpool.tile([128, 1024], BF16, tag=f"wqk{c}", name=f"wqk{c}") for c in range(2)]
            wv_sb = [cpool.tile([128, 256], BF16, tag=f"wv{c}", name=f"wv{c}") for c in range(2)]
            wcv_sb = [cpool.tile([128, 2304], BF16, tag=f"wcv{c}", name=f"wcv{c}") for c in range(2)]
            rel_sb = cpool.tile([128, 126], BF16, tag="rel")
            nc.sync.dma_start(wqk_sb[0][:], wqkt.ap()[0:128, :])
            nc.scalar.dma_start(wqk_sb[1][:], wqkt.ap()[128:256, :])
            nc.sync.dma_start(xp_sb[0][:], xpad.ap()[0:128, :])
            nc.scalar.dma_start(xp_sb[1][:], xpad.ap()[128:256, :])
            nc.scalar.dma_start(rel_sb[:], relw.ap())
            nc.sync.dma_start(xn_sb[0][:], xnat.ap()[0:128, :])
            nc.scalar.dma_start(xn_sb[1][:], xnat.ap()[128:256, :])
            e_sb = cpool.tile([128, 1024], BF16, tag="e_sb")
            nc.scalar.dma_start(e_sb[:], econst.ap())
            for c in range(2):
                nc.scalar.dma_start(wv_sb[c][:], wvt.ap()[128 * c:128 * c + 128, :])
                nc.scalar.dma_start(wcv_sb[c][:], wconvt.ap()[128 * c:128 * c + 128, :])
            wo_sb = []
            for n in range(8):
                t = wopool.tile([32, 256], BF16, name=f"wo{n}")
                nc.scalar.dma_start(t[:], woutt.ap()[32 * n:32 * n + 32, :])
                wo_sb.append(t)
            ones128 = cpool.tile([128, 32], BF16, tag="ones")
            nc.vector.memset(ones128[:], 1.0)
            onescol = cpool.tile([128, 1], BF16, tag="onescol")
            nc.vector.memset(onescol[:], 1.0)

            # interior view of padded x: (128, h 32, w 32), h-stride 34
            def xin(c):
                return xp_sb[c][:].rearrange("p (h w) -> p h w", h=34)[:, 1:33, 1:33]

            qp = [qpool.tile([128, 1024], BF16, name=f"qp{i}") for i in range(8)]
            kp = [kpool.tile([128, 1024], BF16, name=f"kp{i}") for i in range(8)]

            def proj_chunk(m, dest):
                ps = bigps.tile([128, 1024], F32, tag="big", name="projps")
                for c in range(2):
                    for s in range(2):
                        nc.tensor.matmul(
                            ps[:, 512 * s:512 * s + 512],
                            wqk_sb[c][:, 128 * m:128 * m + 128],
                            xin(c)[:, 16 * s:16 * s + 16, :],
                            start=(c == 0), stop=(c == 1),
                        )
                h0 = 2 * (m % 4)
                nc.vector.tensor_copy(dest[h0][0:64, :], ps[0:64, :])
                nc.vector.tensor_copy(dest[h0 + 1][64:128, :], ps[64:128, :])

            def head_tables(n):
                """rel-table matmuls -> stage -> DRAM -> gathers -> sigma copy."""
                base = 0 if n % 2 == 0 else 64
                aw_b = 64 - base
                qn = qp[n][base:base + 64, :]
                qsig = qn.rearrange("p (a b) -> p a b", a=32).transpose([0, 2, 1])
                tps = bigps.tile([128, 1024], F32, tag="big", name="tps")
                for c in range(2):
                    nc.tensor.matmul(
                        tps[0:63, 512 * c:512 * c + 512],
                        rel_sb[base:base + 64, 0:63],
                        qsig[:, 16 * c:16 * c + 16, :],
                        start=True, stop=True,
                    )
                for c in range(2):
                    nc.tensor.matmul(
                        tps[64:127, 512 * c:512 * c + 512],
                        rel_sb[base:base + 64, 63:126],
                        qn[:, 512 * c:512 * c + 512],
                        start=True, stop=True,
                    )
                stg = stpool.tile([128, 1024], BF16, name="stg")
                nc.scalar.copy(stg[:], tps[:])
                nc.sync.dma_start(tdram.ap()[n, :, :], stg[:])
                nc.sync.dma_start(
                    qp[n][aw_b:aw_b + 32, :].rearrange("p (a b) -> p a b", a=32),
                    bass.AP(tdram, n * 131072 + 31 * 1024,
                            [[1024, 32], [-992, 32], [1, 32]]),
                )
                sc = scpool.tile([128, 1024], BF16, name="scr")
                ah_b = aw_b + 32
                nc.sync.dma_start(
                    sc[ah_b:ah_b + 32, :].rearrange("p (a b) -> p a b", a=32),
                    bass.AP(tdram, n * 131072 + 65536 + 31 * 1024,
                            [[1024, 32], [-992, 32], [1, 32]]),
                )
                dst3 = qp[n][ah_b:ah_b + 32, :].rearrange("p (a b) -> p a b", a=32)
                src3 = (sc[ah_b:ah_b + 32, :].rearrange("p (a b) -> p a b", a=32)
                        .transpose([0, 2, 1]))
                nc.vector.tensor_copy(dst3[:, 0:16, :], src3[:, 0:16, :])
                nc.gpsimd.tensor_copy(dst3[:, 16:32, :], src3[:, 16:32, :])
                nc.vector.tensor_copy(kp[n][aw_b:aw_b + 64, :],
                                      e_sb[aw_b:aw_b + 64, :])

            # q-proj, matching k-proj, then that pair's rel tables: heads
            # become attention-ready in increasing order, early
            for m in range(4):
                proj_chunk(m, qp)
                proj_chunk(m + 4, kp)
                head_tables(2 * m)
                head_tables(2 * m + 1)

            # ---- vT projection: vT_all (t, head-major c) + ones col ----
            vto = []
            for j in range(8):
                ps = bigps.tile([128, 256], F32, tag="big", name="vps")
                for c in range(2):
                    nc.tensor.matmul(
                        ps[:], xn_sb[c][:, 128 * j:128 * j + 128], wv_sb[c][:],
                        start=(c == 0), stop=(c == 1),
                    )
                t = vpool.tile([128, 264], BF16, name=f"vto{j}")
                nc.vector.memset(t[:], 1.0)  # cols 33n+32 stay 1.0
                nc.vector.tensor_copy(
                    t[:].rearrange("p (n c) -> p n c", n=8)[:, :, 0:32],
                    ps[:].rearrange("p (n c) -> p n c", n=8),
                )
                vto.append(t)

            def conv_group(o):
                ps = cvps.tile([128, 1024], F32, tag="cv", name="cps")
                for tap in range(9):
                    dy, dx = tap // 3, tap % 3
                    for c in range(2):
                        for hh in range(2):
                            rhs = (xp_sb[c][:]
                                   .rearrange("p (h w) -> p h w", h=34)
                                   [:, dy + 16 * hh:dy + 16 * hh + 16, dx:dx + 32])
                            nc.tensor.matmul(
                                ps[:, 512 * hh:512 * hh + 512],
                                wcv_sb[c][:, 256 * tap + 128 * o:256 * tap + 128 * o + 128],
                                rhs,
                                start=(tap == 0 and c == 0),
                                stop=(tap == 8 and c == 1),
                                skip_group_check=True,
                            )
                osb = opool.tile([128, 1024], F32, name="osb2")
                nc.vector.tensor_copy(osb[:], ps[:])
                nc.sync.dma_start(out_d.ap()[128 * o:128 * o + 128, :], osb[:])

            # conv o=0 fills the PE while rel-table gathers land
            conv_group(0)

            # ---- attention: compute all heads (PV fused with denom row),
            # ---- then normalize (keeps PE off the reciprocal's tail) ----
            att_sb = [apool.tile([32, 1024], BF16, name=f"att{i}") for i in range(8)]
            araw = {}

            sgt = {}
            for grp in range(2):
                for c in range(2):
                    sgt[(grp, c)] = sspool.tile([97, 512], F32, tag=f"sg{grp}{c}",
                                                name=f"sg{grp}{c}", bufs=1)

            def group_recip(grp):
                recf = rpool.tile([97, 1024], F32, tag="recf", name="recf")
                rec = rpool.tile([97, 1024], BF16, name="rec")
                for c in range(2):
                    nc.vector.reciprocal_approx_fast(
                        out=recf[0:97, 512 * c:512 * c + 512],
                        in_=sgt[(grp, c)][:])
                    nc.vector.tensor_copy(rec[0:97, 512 * c:512 * c + 512],
                                          recf[0:97, 512 * c:512 * c + 512])
                return rec

            def group_norm(grp, rec):
                for g in range(4):
                    n = 4 * grp + g
                    for c in range(2):
                        bc = abps.tile([32, 512], F32, tag="ab", name="bc")
                        nc.tensor.matmul(
                            bc[:], ones128[32 * g:32 * g + 1, 0:32],
                            rec[32 * g:32 * g + 1, 512 * c:512 * c + 512],
                            start=True, stop=True,
                            tile_position=(32 * g, 0),
                        )
                        bcs = sspool.tile([32, 512], F32, tag="bcs", name="bcs")
                        nc.scalar.copy(bcs[:], bc[:])
                        nc.vector.tensor_tensor(
                            att_sb[n][:, 512 * c:512 * c + 512],
                            araw[(n, c)][0:32, :], bcs[:], op=ALU.mult,
                        )
            for n in range(8):
                expt = []
                for j in range(8):
                    lt = bigps.tile([128, 1024], F32, tag="big", name="lt")
                    for c in range(2):
                        nc.tensor.matmul(
                            lt[:, 512 * c:512 * c + 512],
                            kp[n][:, 128 * j:128 * j + 128],
                            qp[n][:, 512 * c:512 * c + 512],
                            start=True, stop=True,
                        )
                    et = epool.tile([128, 1024], BF16, name="et")
                    nc.scalar.activation(et[:], lt[:], AF.Exp)
                    expt.append(et)
                aps2 = [abps.tile([33, 512], F32, tag="ab", name=f"aps{c}")
                        for c in range(2)]
                for j in range(8):
                    for c in range(2):
                        nc.tensor.matmul(
                            aps2[c][:],
                            vto[j][:, 33 * n:33 * n + 33],
                            expt[j][:, 512 * c:512 * c + 512],
                            start=(j == 0), stop=(j == 7),
                            skip_group_check=True,
                        )
                for c in range(2):
                    ar = sspool.tile([33, 512], F32, tag="araw",
                                     name="araw", bufs=16)
                    nc.vector.tensor_copy(ar[:], aps2[c][:])
                    araw[(n, c)] = ar
                    nc.sync.dma_start(sgt[(n // 4, c)][32 * (n % 4):32 * (n % 4) + 1, :],
                                      ar[32:33, :])
                if n == 3:
                    rec0 = group_recip(0)
                if n == 4:
                    group_norm(0, rec0)


            conv_group(1)
            group_norm(1, group_recip(1))

            # ---- attn out-projection -> out rows 256..511 ----
            for o in range(2):
                ps = bigps.tile([128, 1024], F32, tag="big", name="pout")
                for n in range(8):
                    for c in range(2):
                        nc.tensor.matmul(
                            ps[:, 512 * c:512 * c + 512],
                            wo_sb[n][:, 128 * o:128 * o + 128],
                            att_sb[n][:, 512 * c:512 * c + 512],
                            start=(n == 0), stop=(n == 7),
                            skip_group_check=True,
                        )
                osb = opool.tile([128, 1024], F32, name="osb")
                nc.vector.tensor_copy(osb[:], ps[:])
                nc.sync.dma_start(out_d.ap()[256 + 128 * o:384 + 128 * o, :], osb[:])


    nc.compile()
    return nc


_NC_CACHE = None


def kernel(x, w_qkv, w_conv, w_out, key_rel_h, key_rel_w):
    global _NC_CACHE, LAST_RESULT
    x = np.asarray(x, np.float32)
    w_qkv = np.asarray(w_qkv, np.float32)
    w_conv = np.asarray(w_conv, np.float32)
    w_out = np.asarray(w_out, np.float32)
    key_rel_h = np.asarray(key_rel_h, np.float32)
    key_rel_w = np.asarray(key_rel_w, np.float32)

    wq = w_qkv.copy()
    wq[:DK] *= DKH ** -0.5
    wqkt = _bf(wq[:1024].T)                      # (256, 1024)
    wvt = _bf(wq[1024:].T)                       # (256, 256)
    wconvt = _bf(w_conv.transpose(1, 2, 3, 0).reshape(256, 9 * 256))
    woutt = _bf(w_out.T)
    rel2 = np.concatenate([key_rel_w, key_rel_h], axis=1)  # (64, 126)
    relw = _bf(np.concatenate([rel2, rel2], axis=0))       # (128, 126)
    t = np.arange(L)
    ew = (t[None, :] // 32 == np.arange(32)[:, None]).astype(np.float32)
    eh = (t[None, :] % 32 == np.arange(32)[:, None]).astype(np.float32)
    e64 = np.concatenate([ew, eh], axis=0)
    econst = _bf(np.concatenate([e64, e64], axis=0))       # (128, 1024)

    shared = dict(wqkt=wqkt, wvt=wvt, wconvt=wconvt, woutt=woutt,
                  relw=relw, econst=econst)
    in_maps = []
    for b in range(B):
        xp = np.zeros((256, 34, 34), np.float32)
        xp[:, 1:33, 1:33] = x[b]
        in_maps.append(dict(shared, xpad=_bf(xp.reshape(256, 1156)),
                            xnat=_bf(x[b].reshape(256, 1024))))

    if _NC_CACHE is None:
        _NC_CACHE = build()
    res = run_bass_kernel_spmd(_NC_CACHE, in_maps, core_ids=list(range(8)),
                               trace=TRACE, **TRACE_KW)
    LAST_RESULT = res
    out = np.stack([res.results[i]["out"] for i in range(B)])
    return out.reshape(B, 512, H, W).astype(np.float32)


# revision 16
# speedup vs baseline: 1.2842x; 1.0248x over previous
"""AAConv2d (attention-augmented conv) Trainium2 kernel.

Data-parallel over batch: 8 images -> 8 NeuronCores, no collectives.
Per core: qkv projection, 8-head attention with relative-position logits
folded into the QK matmul as extra contraction rows, softmax (no max-sub;
logits are small), PV, out-projection, and a 3x3 conv via 9 shifted
matmuls on a zero-padded input. All matmul operands bf16, PSUM f32.

Layout notes (per head n, base = 0 for even n / 64 for odd n to match the
partition placement of the projection PSUM output):
  Qp[n] (128, 1024): rows base..base+63 = Q_n, the other 64 rows hold
      [Aw (32); Ah (32)] - the gathered relative-logit tables.
  Kp[n] (128, 1024): rows base.. = K_n, other 64 rows = [Ew; Eh] consts.
  logits^T tile (t, s) = Kp-chunk^T @ Qp, includes QK + w_rel + h_rel.
Softmax denominators: per 4-head group, ones^T @ exp matmuls stack the
sums at partition rows {0,32,64,96} of one PSUM tile so a single
reciprocal covers 4 heads (DVE reciprocal cost is per-free-element).
"""
import numpy as np
import ml_dtypes

import concourse.bass as bass
import concourse.tile as tile
from concourse import bacc, mybir
from concourse.bass_utils import run_bass_kernel_spmd

F32 = mybir.dt.float32
BF16 = mybir.dt.bfloat16
AF = mybir.ActivationFunctionType
ALU = mybir.AluOpType

B, CIN, H, W = 8, 256, 32, 32
L = H * W
DK, DV, NH = 512, 256, 8
DKH, DVH = DK // NH, DV // NH

TRACE = False
TRACE_KW = {}
LAST_RESULT = None


def _bf(a):
    return np.ascontiguousarray(a).astype(ml_dtypes.bfloat16)


def build():
    nc = bacc.Bacc("TRN2", target_bir_lowering=False, debug=False, num_devices=8)

    xpad = nc.dram_tensor("xpad", [256, 1156], BF16, kind="ExternalInput")
    xnat = nc.dram_tensor("xnat", [256, 1024], BF16, kind="ExternalInput")
    wqkt = nc.dram_tensor("wqkt", [256, 1024], BF16, kind="ExternalInput")
    wvt = nc.dram_tensor("wvt", [256, 256], BF16, kind="ExternalInput")
    wconvt = nc.dram_tensor("wconvt", [256, 2304], BF16, kind="ExternalInput")
    woutt = nc.dram_tensor("woutt", [256, 256], BF16, kind="ExternalInput")
    relw = nc.dram_tensor("relw", [128, 126], BF16, kind="ExternalInput")
    econst = nc.dram_tensor("econst", [128, 1024], BF16, kind="ExternalInput")
    out_d = nc.dram_tensor("out", [512, 1024], F32, kind="ExternalOutput")
    tdram = nc.dram_tensor("tdram", [8, 128, 1024], BF16)  # rel-table scratch

    with tile.TileContext(nc) as tc:
        with (
            tc.tile_pool(name="const", bufs=1) as cpool,
            tc.tile_pool(name="qp", bufs=1) as qpool,
            tc.tile_pool(name="kp", bufs=1) as kpool,
            tc.tile_pool(name="vto", bufs=1) as vpool,
            tc.tile_pool(name="attn", bufs=1) as apool,
            tc.tile_pool(name="wo", bufs=1) as wopool,
            tc.tile_pool(name="expt", bufs=12) as epool,
            tc.tile_pool(name="stage", bufs=2) as stpool,
            tc.tile_pool(name="scratch", bufs=2) as scpool,
            tc.tile_pool(name="outsb", bufs=4) as opool,
            tc.tile_pool(name="small_sb", bufs=4) as sspool,
            tc.tile_pool(name="rec_sb", bufs=2) as rpool,
            tc.tile_pool(name="bigps", bufs=2, space="PSUM") as bigps,
            tc.tile_pool(name="convps", bufs=1, space="PSUM") as cvps,
            tc.tile_pool(name="attbc", bufs=2, space="PSUM") as abps,
        ):
            # ---- load inputs: latency-critical ones on HWDGE (sync),
            # ---- bulky late-use weights on SWDGE (gpsimd) in parallel ----
            xp_sb = [cpool.tile([128, 1156], BF16, tag=f"xp{c}", name=f"xp{c}") for c in range(2)]
            xn_sb = [cpool.tile([128, 1024], BF16, tag=f"xn{c}", name=f"xn{c}") for c in range(2)]
            wqk_sb = [cpool.tile([128, 1024], BF16, tag=f"wqk{c}", name=f"wqk{c}") for c in range(2)]
            wv_sb = [cpool.tile([128, 256], BF16, tag=f"wv{c}", name=f"wv{c}") for c in range(2)]
            wcv_sb = [cpool.tile([128, 2304], BF16, tag=f"wcv{c}", name=f"wcv{c}") for c in range(2)]
            rel_sb = cpool.tile([128, 126], BF16, tag="rel")
            nc.sync.dma_start(wqk_sb[0][:], wqkt.ap()[0:128, :])
            nc.scalar.dma_start(wqk_sb[1][:], wqkt.ap()[128:256, :])
            nc.sync.dma_start(xp_sb[0][:], xpad.ap()[0:128, :])
            nc.scalar.dma_start(xp_sb[1][:], xpad.ap()[128:256, :])
            nc.scalar.dma_start(rel_sb[:], relw.ap())
            nc.sync.dma_start(xn_sb[0][:], xnat.ap()[0:128, :])
            nc.scalar.dma_start(xn_sb[1][:], xnat.ap()[128:256, :])
            e_sb = cpool.tile([128, 1024], BF16, tag="e_sb")
            nc.scalar.dma_start(e_sb[:], econst.ap())
            for c in range(2):
                nc.scalar.dma_start(wv_sb[c][:], wvt.ap()[128 * c:128 * c + 128, :])
                nc.scalar.dma_start(wcv_sb[c][:], wconvt.ap()[128 * c:128 * c + 128, :])
            wo_sb = []
            for n in range(8):
                t = wopool.tile([32, 256], BF16, name=f"wo{n}")
                nc.scalar.dma_start(t[:], woutt.ap()[32 * n:32 * n + 32, :])
                wo_sb.append(t)
            ones128 = cpool.tile([128, 32], BF16, tag="ones")
            nc.vector.memset(ones128[:], 1.0)
            onescol = cpool.tile([128, 1], BF16, tag="onescol")
            nc.vector.memset(onescol[:], 1.0)

            # interior view of padded x: (128, h 32, w 32), h-stride 34
            def xin(c):
                return xp_sb[c][:].rearrange("p (h w) -> p h w", h=34)[:, 1:33, 1:33]

            qp = [qpool.tile([128, 1024], BF16, name=f"qp{i}") for i in range(8)]
            kp = [kpool.tile([128, 1024], BF16, name=f"kp{i}") for i in range(8)]

            def proj_chunk(m, dest):
                ps = bigps.tile([128, 1024], F32, tag="big", name="projps")
                for c in range(2):
                    for s in range(2):
                        nc.tensor.matmul(
                            ps[:, 512 * s:512 * s + 512],
                            wqk_sb[c][:, 128 * m:128 * m + 128],
                            xin(c)[:, 16 * s:16 * s + 16, :],
                            start=(c == 0), stop=(c == 1),
                        )
                h0 = 2 * (m % 4)
                nc.vector.tensor_copy(dest[h0][0:64, :], ps[0:64, :])
                nc.vector.tensor_copy(dest[h0 + 1][64:128, :], ps[64:128, :])

            def head_tables(n):
                """rel-table matmuls -> stage -> DRAM -> gathers -> sigma copy."""
                base = 0 if n % 2 == 0 else 64
                aw_b = 64 - base
                qn = qp[n][base:base + 64, :]
                qsig = qn.rearrange("p (a b) -> p a b", a=32).transpose([0, 2, 1])
                tps = bigps.tile([128, 1024], F32, tag="big", name="tps")
                for c in range(2):
                    nc.tensor.matmul(
                        tps[0:63, 512 * c:512 * c + 512],
                        rel_sb[base:base + 64, 0:63],
                        qsig[:, 16 * c:16 * c + 16, :],
                        start=True, stop=True,
                    )
                for c in range(2):
                    nc.tensor.matmul(
                        tps[64:127, 512 * c:512 * c + 512],
                        rel_sb[base:base + 64, 63:126],
                        qn[:, 512 * c:512 * c + 512],
                        start=True, stop=True,
                    )
                stg = stpool.tile([128, 1024], BF16, name="stg")
                nc.scalar.copy(stg[:], tps[:])
                nc.sync.dma_start(tdram.ap()[n, :, :], stg[:])
                nc.sync.dma_start(
                    qp[n][aw_b:aw_b + 32, :].rearrange("p (a b) -> p a b", a=32),
                    bass.AP(tdram, n * 131072 + 31 * 1024,
                            [[1024, 32], [-992, 32], [1, 32]]),
                )
                sc = scpool.tile([128, 1024], BF16, name="scr")
                ah_b = aw_b + 32
                nc.sync.dma_start(
                    sc[ah_b:ah_b + 32, :].rearrange("p (a b) -> p a b", a=32),
                    bass.AP(tdram, n * 131072 + 65536 + 31 * 1024,
                            [[1024, 32], [-992, 32], [1, 32]]),
                )
                dst3 = qp[n][ah_b:ah_b + 32, :].rearrange("p (a b) -> p a b", a=32)
                src3 = (sc[ah_b:ah_b + 32, :].rearrange("p (a b) -> p a b", a=32)
                        .transpose([0, 2, 1]))
                nc.vector.tensor_copy(dst3[:, 0:16, :], src3[:, 0:16, :])
                nc.gpsimd.tensor_copy(dst3[:, 16:32, :], src3[:, 16:32, :])
                nc.vector.tensor_copy(kp[n][aw_b:aw_b + 64, :],
                                      e_sb[aw_b:aw_b + 64, :])

            # q-proj, matching k-proj; tables one pair behind so PE always
            # has projection work while DVE copies / DMAs land
            proj_chunk(0, qp); proj_chunk(4, kp)
            proj_chunk(1, qp); proj_chunk(5, kp)
            head_tables(0); head_tables(1)
            proj_chunk(2, qp); proj_chunk(6, kp)
            head_tables(2); head_tables(3)
            proj_chunk(3, qp); proj_chunk(7, kp)
            head_tables(4); head_tables(5)

            # ---- vT projection: vT_all (t, head-major c) + ones col ----
            vto = []
            for j in range(8):
                ps = bigps.tile([128, 256], F32, tag="big", name="vps")
                for c in range(2):
                    nc.tensor.matmul(
                        ps[:], xn_sb[c][:, 128 * j:128 * j + 128], wv_sb[c][:],
                        start=(c == 0), stop=(c == 1),
                    )
                t = vpool.tile([128, 264], BF16, name=f"vto{j}")
                nc.vector.memset(t[:], 1.0)  # cols 33n+32 stay 1.0
                nc.vector.tensor_copy(
                    t[:].rearrange("p (n c) -> p n c", n=8)[:, :, 0:32],
                    ps[:].rearrange("p (n c) -> p n c", n=8),
                )
                vto.append(t)
            head_tables(6); head_tables(7)

            def conv_group(o):
                ps = cvps.tile([128, 1024], F32, tag="cv", name="cps")
                for tap in range(9):
                    dy, dx = tap // 3, tap % 3
                    for c in range(2):
                        for hh in range(2):
                            rhs = (xp_sb[c][:]
                                   .rearrange("p (h w) -> p h w", h=34)
                                   [:, dy + 16 * hh:dy + 16 * hh + 16, dx:dx + 32])
                            nc.tensor.matmul(
                                ps[:, 512 * hh:512 * hh + 512],
                                wcv_sb[c][:, 256 * tap + 128 * o:256 * tap + 128 * o + 128],
                                rhs,
                                start=(tap == 0 and c == 0),
                                stop=(tap == 8 and c == 1),
                                skip_group_check=True,
                            )
                osb = opool.tile([128, 1024], F32, name="osb2")
                nc.vector.tensor_copy(osb[:], ps[:])
                nc.sync.dma_start(out_d.ap()[128 * o:128 * o + 128, :], osb[:])

            # conv o=0 fills the PE while rel-table gathers land
            conv_group(0)

            # ---- attention: compute all heads (PV fused with denom row),
            # ---- then normalize (keeps PE off the reciprocal's tail) ----
            att_sb = [apool.tile([32, 1024], BF16, name=f"att{i}") for i in range(8)]
            araw = {}

            sgt = {}
            for grp in range(2):
                for c in range(2):
                    sgt[(grp, c)] = sspool.tile([97, 512], F32, tag=f"sg{grp}{c}",
                                                name=f"sg{grp}{c}", bufs=1)

            def group_recip(grp):
                recf = rpool.tile([97, 1024], F32, tag="recf", name="recf")
                rec = rpool.tile([97, 1024], BF16, name="rec")
                for c in range(2):
                    nc.vector.reciprocal_approx_fast(
                        out=recf[0:97, 512 * c:512 * c + 512],
                        in_=sgt[(grp, c)][:])
                    nc.vector.tensor_copy(rec[0:97, 512 * c:512 * c + 512],
                                          recf[0:97, 512 * c:512 * c + 512])
                return rec

            def group_norm(grp, rec):
                for g in range(4):
                    n = 4 * grp + g
                    for c in range(2):
                        bc = abps.tile([32, 512], F32, tag="ab", name="bc")
                        nc.tensor.matmul(
                            bc[:], ones128[32 * g:32 * g + 1, 0:32],
                            rec[32 * g:32 * g + 1, 512 * c:512 * c + 512],
                            start=True, stop=True,
                            tile_position=(32 * g, 0),
                        )
                        bcs = sspool.tile([32, 512], F32, tag="bcs", name="bcs")
                        nc.scalar.copy(bcs[:], bc[:])
                        nc.vector.tensor_tensor(
                            att_sb[n][:, 512 * c:512 * c + 512],
                            araw[(n, c)][0:32, :], bcs[:], op=ALU.mult,
                        )
            for n in range(8):
                expt = []
                for j in range(8):
                    lt = bigps.tile([128, 1024], F32, tag="big", name="lt")
                    for c in range(2):
                        nc.tensor.matmul(
                            lt[:, 512 * c:512 * c + 512],
                            kp[n][:, 128 * j:128 * j + 128],
                            qp[n][:, 512 * c:512 * c + 512],
                            start=True, stop=True,
                        )
                    et = epool.tile([128, 1024], BF16, name="et")
                    nc.scalar.activation(et[:], lt[:], AF.Exp)
                    expt.append(et)
                aps2 = [abps.tile([33, 512], F32, tag="ab", name=f"aps{c}")
                        for c in range(2)]
                for j in range(8):
                    for c in range(2):
                        nc.tensor.matmul(
                            aps2[c][:],
                            vto[j][:, 33 * n:33 * n + 33],
                            expt[j][:, 512 * c:512 * c + 512],
                            start=(j == 0), stop=(j == 7),
                            skip_group_check=True,
                        )
                for c in range(2):
                    ar = sspool.tile([33, 512], F32, tag="araw",
                                     name="araw", bufs=16)
                    nc.vector.tensor_copy(ar[:], aps2[c][:])
                    araw[(n, c)] = ar
                    nc.sync.dma_start(sgt[(n // 4, c)][32 * (n % 4):32 * (n % 4) + 1, :],
                                      ar[32:33, :])
                if n == 3:
                    rec0 = group_recip(0)
                if n == 4:
                    group_norm(0, rec0)


            group_norm(1, group_recip(1))
            conv_group(1)

            # ---- attn out-projection -> out rows 256..511 ----
            for o in range(2):
                ps = bigps.tile([128, 1024], F32, tag="big", name="pout")
                for n in range(8):
                    for c in range(2):
                        nc.tensor.matmul(
                            ps[:, 512 * c:512 * c + 512],
                            wo_sb[n][:, 128 * o:128 * o + 128],
                            att_sb[n][:, 512 * c:512 * c + 512],
                            start=(n == 0), stop=(n == 7),
                            skip_group_check=True,
                        )
                osb = opool.tile([128, 1024], F32, name="osb")
                nc.vector.tensor_copy(osb[:], ps[:])
                nc.sync.dma_start(out_d.ap()[256 + 128 * o:384 + 128 * o, :], osb[:])


    nc.compile()
    return nc


_NC_CACHE = None


def kernel(x, w_qkv, w_conv, w_out, key_rel_h, key_rel_w):
    global _NC_CACHE, LAST_RESULT
    x = np.asarray(x, np.float32)
    w_qkv = np.asarray(w_qkv, np.float32)
    w_conv = np.asarray(w_conv, np.float32)
    w_out = np.asarray(w_out, np.float32)
    key_rel_h = np.asarray(key_rel_h, np.float32)
    key_rel_w = np.asarray(key_rel_w, np.float32)

    wq = w_qkv.copy()
    wq[:DK] *= DKH ** -0.5
    wqkt = _bf(wq[:1024].T)                      # (256, 1024)
    wvt = _bf(wq[1024:].T)                       # (256, 256)
    wconvt = _bf(w_conv.transpose(1, 2, 3, 0).reshape(256, 9 * 256))
    woutt = _bf(w_out.T)
    rel2 = np.concatenate([key_rel_w, key_rel_h], axis=1)  # (64, 126)
    relw = _bf(np.concatenate([rel2, rel2], axis=0))       # (128, 126)
    t = np.arange(L)
    ew = (t[None, :] // 32 == np.arange(32)[:, None]).astype(np.float32)
    eh = (t[None, :] % 32 == np.arange(32)[:, None]).astype(np.float32)
    e64 = np.concatenate([ew, eh], axis=0)
    econst = _bf(np.concatenate([e64, e64], axis=0))       # (128, 1024)

    shared = dict(wqkt=wqkt, wvt=wvt, wconvt=wconvt, woutt=woutt,
                  relw=relw, econst=econst)
    in_maps = []
    for b in range(B):
        xp = np.zeros((256, 34, 34), np.float32)
        xp[:, 1:33, 1:33] = x[b]
        in_maps.append(dict(shared, xpad=_bf(xp.reshape(256, 1156)),
                            xnat=_bf(x[b].reshape(256, 1024))))

    if _NC_CACHE is None:
        _NC_CACHE = build()
    res = run_bass_kernel_spmd(_NC_CACHE, in_maps, core_ids=list(range(8)),
                               trace=TRACE, **TRACE_KW)
    LAST_RESULT = res
    out = np.stack([res.results[i]["out"] for i in range(B)])
    return out.reshape(B, 512, H, W).astype(np.float32)


# revision 17
# speedup vs baseline: 1.3156x; 1.0245x over previous
"""AAConv2d (attention-augmented conv) Trainium2 kernel.

Data-parallel over batch: 8 images -> 8 NeuronCores, no collectives.
Per core: qkv projection, 8-head attention with relative-position logits
folded into the QK matmul as extra contraction rows, softmax (no max-sub;
logits are small), PV, out-projection, and a 3x3 conv via 9 shifted
matmuls on a zero-padded input. All matmul operands bf16, PSUM f32.

Layout notes (per head n, base = 0 for even n / 64 for odd n to match the
partition placement of the projection PSUM output):
  Qp[n] (128, 1024): rows base..base+63 = Q_n, the other 64 rows hold
      [Aw (32); Ah (32)] - the gathered relative-logit tables.
  Kp[n] (128, 1024): rows base.. = K_n, other 64 rows = [Ew; Eh] consts.
  logits^T tile (t, s) = Kp-chunk^T @ Qp, includes QK + w_rel + h_rel.
Softmax denominators: per 4-head group, ones^T @ exp matmuls stack the
sums at partition rows {0,32,64,96} of one PSUM tile so a single
reciprocal covers 4 heads (DVE reciprocal cost is per-free-element).
"""
import numpy as np
import ml_dtypes

import concourse.bass as bass
import concourse.tile as tile
from concourse import bacc, mybir
from concourse.bass_utils import run_bass_kernel_spmd

F32 = mybir.dt.float32
BF16 = mybir.dt.bfloat16
AF = mybir.ActivationFunctionType
ALU = mybir.AluOpType

B, CIN, H, W = 8, 256, 32, 32
L = H * W
DK, DV, NH = 512, 256, 8
DKH, DVH = DK // NH, DV // NH

TRACE = False
TRACE_KW = {}
LAST_RESULT = None


def _bf(a):
    return np.ascontiguousarray(a).astype(ml_dtypes.bfloat16)


def build():
    nc = bacc.Bacc("TRN2", target_bir_lowering=False, debug=False, num_devices=8)

    xpad = nc.dram_tensor("xpad", [256, 1156], BF16, kind="ExternalInput")
    xnat = nc.dram_tensor("xnat", [256, 1024], BF16, kind="ExternalInput")
    wqkt = nc.dram_tensor("wqkt", [256, 1024], BF16, kind="ExternalInput")
    wvt = nc.dram_tensor("wvt", [256, 256], BF16, kind="ExternalInput")
    wconvt = nc.dram_tensor("wconvt", [256, 2304], BF16, kind="ExternalInput")
    woutt = nc.dram_tensor("woutt", [256, 256], BF16, kind="ExternalInput")
    relw = nc.dram_tensor("relw", [128, 126], BF16, kind="ExternalInput")
    econst = nc.dram_tensor("econst", [128, 1024], BF16, kind="ExternalInput")
    out_d = nc.dram_tensor("out", [512, 1024], F32, kind="ExternalOutput")
    tdram = nc.dram_tensor("tdram", [8, 128, 1024], BF16)  # rel-table scratch

    with tile.TileContext(nc) as tc:
        with (
            tc.tile_pool(name="const", bufs=1) as cpool,
            tc.tile_pool(name="qp", bufs=1) as qpool,
            tc.tile_pool(name="kp", bufs=1) as kpool,
            tc.tile_pool(name="vto", bufs=1) as vpool,
            tc.tile_pool(name="attn", bufs=1) as apool,
            tc.tile_pool(name="wo", bufs=1) as wopool,
            tc.tile_pool(name="expt", bufs=12) as epool,
            tc.tile_pool(name="stage", bufs=2) as stpool,
            tc.tile_pool(name="scratch", bufs=2) as scpool,
            tc.tile_pool(name="outsb", bufs=4) as opool,
            tc.tile_pool(name="small_sb", bufs=4) as sspool,
            tc.tile_pool(name="rec_sb", bufs=2) as rpool,
            tc.tile_pool(name="bigps", bufs=2, space="PSUM") as bigps,
            tc.tile_pool(name="convps", bufs=1, space="PSUM") as cvps,
            tc.tile_pool(name="attbc", bufs=2, space="PSUM") as abps,
        ):
            # ---- load inputs: latency-critical ones on HWDGE (sync),
            # ---- bulky late-use weights on SWDGE (gpsimd) in parallel ----
            xp_sb = [cpool.tile([128, 1156], BF16, tag=f"xp{c}", name=f"xp{c}") for c in range(2)]
            xn_sb = [cpool.tile([128, 1024], BF16, tag=f"xn{c}", name=f"xn{c}") for c in range(2)]
            wqk_sb = [cpool.tile([128, 1024], BF16, tag=f"wqk{c}", name=f"wqk{c}") for c in range(2)]
            wv_sb = [cpool.tile([128, 256], BF16, tag=f"wv{c}", name=f"wv{c}") for c in range(2)]
            wcv_sb = [cpool.tile([128, 2304], BF16, tag=f"wcv{c}", name=f"wcv{c}") for c in range(2)]
            rel_sb = cpool.tile([128, 126], BF16, tag="rel")
            nc.sync.dma_start(wqk_sb[0][:], wqkt.ap()[0:128, :])
            nc.scalar.dma_start(wqk_sb[1][:], wqkt.ap()[128:256, :])
            nc.sync.dma_start(xp_sb[0][:], xpad.ap()[0:128, :])
            nc.scalar.dma_start(xp_sb[1][:], xpad.ap()[128:256, :])
            nc.scalar.dma_start(rel_sb[:], relw.ap())
            nc.sync.dma_start(xn_sb[0][:], xnat.ap()[0:128, :])
            nc.scalar.dma_start(xn_sb[1][:], xnat.ap()[128:256, :])
            e_sb = cpool.tile([128, 1024], BF16, tag="e_sb")
            nc.scalar.dma_start(e_sb[:], econst.ap())
            for c in range(2):
                nc.scalar.dma_start(wv_sb[c][:], wvt.ap()[128 * c:128 * c + 128, :])
                nc.scalar.dma_start(wcv_sb[c][:], wconvt.ap()[128 * c:128 * c + 128, :])
            wo_sb = []
            for n in range(8):
                t = wopool.tile([32, 256], BF16, name=f"wo{n}")
                nc.scalar.dma_start(t[:], woutt.ap()[32 * n:32 * n + 32, :])
                wo_sb.append(t)
            ones128 = cpool.tile([128, 32], BF16, tag="ones")
            nc.vector.memset(ones128[:], 1.0)
            onescol = cpool.tile([128, 1], BF16, tag="onescol")
            nc.vector.memset(onescol[:], 1.0)

            # interior view of padded x: (128, h 32, w 32), h-stride 34
            def xin(c):
                return xp_sb[c][:].rearrange("p (h w) -> p h w", h=34)[:, 1:33, 1:33]

            qp = [qpool.tile([128, 1024], BF16, name=f"qp{i}") for i in range(8)]
            kp = [kpool.tile([128, 1024], BF16, name=f"kp{i}") for i in range(8)]

            def proj_chunk(m, dest):
                ps = bigps.tile([128, 1024], F32, tag="big", name="projps")
                for c in range(2):
                    for s in range(2):
                        nc.tensor.matmul(
                            ps[:, 512 * s:512 * s + 512],
                            wqk_sb[c][:, 128 * m:128 * m + 128],
                            xin(c)[:, 16 * s:16 * s + 16, :],
                            start=(c == 0), stop=(c == 1),
                        )
                h0 = 2 * (m % 4)
                nc.vector.tensor_copy(dest[h0][0:64, :], ps[0:64, :])
                nc.vector.tensor_copy(dest[h0 + 1][64:128, :], ps[64:128, :])

            def head_tables(n):
                """rel-table matmuls -> stage -> DRAM -> gathers -> sigma copy."""
                base = 0 if n % 2 == 0 else 64
                aw_b = 64 - base
                qn = qp[n][base:base + 64, :]
                qsig = qn.rearrange("p (a b) -> p a b", a=32).transpose([0, 2, 1])
                tps = bigps.tile([128, 1024], F32, tag="big", name="tps")
                for c in range(2):
                    nc.tensor.matmul(
                        tps[0:63, 512 * c:512 * c + 512],
                        rel_sb[base:base + 64, 0:63],
                        qsig[:, 16 * c:16 * c + 16, :],
                        start=True, stop=True,
                    )
                for c in range(2):
                    nc.tensor.matmul(
                        tps[64:127, 512 * c:512 * c + 512],
                        rel_sb[base:base + 64, 63:126],
                        qn[:, 512 * c:512 * c + 512],
                        start=True, stop=True,
                    )
                stg = stpool.tile([128, 1024], BF16, name="stg")
                nc.scalar.copy(stg[:], tps[:])
                nc.sync.dma_start(tdram.ap()[n, :, :], stg[:])
                nc.sync.dma_start(
                    qp[n][aw_b:aw_b + 32, :].rearrange("p (a b) -> p a b", a=32),
                    bass.AP(tdram, n * 131072 + 31 * 1024,
                            [[1024, 32], [-992, 32], [1, 32]]),
                )
                sc = scpool.tile([128, 1024], BF16, name="scr")
                ah_b = aw_b + 32
                nc.sync.dma_start(
                    sc[ah_b:ah_b + 32, :].rearrange("p (a b) -> p a b", a=32),
                    bass.AP(tdram, n * 131072 + 65536 + 31 * 1024,
                            [[1024, 32], [-992, 32], [1, 32]]),
                )
                dst3 = qp[n][ah_b:ah_b + 32, :].rearrange("p (a b) -> p a b", a=32)
                src3 = (sc[ah_b:ah_b + 32, :].rearrange("p (a b) -> p a b", a=32)
                        .transpose([0, 2, 1]))
                nc.vector.tensor_copy(dst3[:, 0:16, :], src3[:, 0:16, :])
                nc.gpsimd.tensor_copy(dst3[:, 16:32, :], src3[:, 16:32, :])
                nc.vector.tensor_copy(kp[n][aw_b:aw_b + 64, :],
                                      e_sb[aw_b:aw_b + 64, :])

            # q-proj, matching k-proj; tables one pair behind so PE always
            # has projection work while DVE copies / DMAs land
            proj_chunk(0, qp); proj_chunk(4, kp)
            proj_chunk(1, qp); proj_chunk(5, kp)
            head_tables(0); head_tables(1)
            proj_chunk(2, qp); proj_chunk(6, kp)
            head_tables(2); head_tables(3)
            proj_chunk(3, qp); proj_chunk(7, kp)
            head_tables(4); head_tables(5)

            # ---- vT projection: vT_all (t, head-major c) + ones col ----
            vto = []
            for j in range(8):
                ps = bigps.tile([128, 256], F32, tag="big", name="vps")
                for c in range(2):
                    nc.tensor.matmul(
                        ps[:], xn_sb[c][:, 128 * j:128 * j + 128], wv_sb[c][:],
                        start=(c == 0), stop=(c == 1),
                    )
                t = vpool.tile([128, 264], BF16, name=f"vto{j}")
                nc.vector.memset(t[:], 1.0)  # cols 33n+32 stay 1.0
                nc.vector.tensor_copy(
                    t[:].rearrange("p (n c) -> p n c", n=8)[:, :, 0:32],
                    ps[:].rearrange("p (n c) -> p n c", n=8),
                )
                vto.append(t)
            head_tables(6); head_tables(7)

            def conv_taps(o, ps, lo, hi):
                for tap in range(lo, hi):
                    dy, dx = tap // 3, tap % 3
                    for c in range(2):
                        for hh in range(2):
                            rhs = (xp_sb[c][:]
                                   .rearrange("p (h w) -> p h w", h=34)
                                   [:, dy + 16 * hh:dy + 16 * hh + 16, dx:dx + 32])
                            nc.tensor.matmul(
                                ps[:, 512 * hh:512 * hh + 512],
                                wcv_sb[c][:, 256 * tap + 128 * o:256 * tap + 128 * o + 128],
                                rhs,
                                start=(tap == 0 and c == 0),
                                stop=(tap == 8 and c == 1),
                                skip_group_check=True,
                            )

            def conv_finish(o, ps):
                osb = opool.tile([128, 1024], F32, name="osb2")
                nc.vector.tensor_copy(osb[:], ps[:])
                nc.sync.dma_start(out_d.ap()[128 * o:128 * o + 128, :], osb[:])

            def conv_group(o):
                ps = cvps.tile([128, 1024], F32, tag="cv", name="cps")
                conv_taps(o, ps, 0, 9)
                conv_finish(o, ps)

            # conv o=0 fills the PE while rel-table gathers land
            conv_group(0)

            # ---- attention: compute all heads (PV fused with denom row),
            # ---- then normalize (keeps PE off the reciprocal's tail) ----
            att_sb = [apool.tile([32, 1024], BF16, name=f"att{i}") for i in range(8)]
            araw = {}

            sgt = {}
            for grp in range(2):
                for c in range(2):
                    sgt[(grp, c)] = sspool.tile([97, 512], F32, tag=f"sg{grp}{c}",
                                                name=f"sg{grp}{c}", bufs=1)

            def group_recip(grp):
                recf = rpool.tile([97, 1024], F32, tag="recf", name="recf")
                rec = rpool.tile([97, 1024], BF16, name="rec")
                for c in range(2):
                    nc.vector.reciprocal_approx_fast(
                        out=recf[0:97, 512 * c:512 * c + 512],
                        in_=sgt[(grp, c)][:])
                    nc.vector.tensor_copy(rec[0:97, 512 * c:512 * c + 512],
                                          recf[0:97, 512 * c:512 * c + 512])
                return rec

            def group_norm(grp, rec):
                for g in range(4):
                    n = 4 * grp + g
                    for c in range(2):
                        bc = abps.tile([32, 512], F32, tag="ab", name="bc")
                        nc.tensor.matmul(
                            bc[:], ones128[32 * g:32 * g + 1, 0:32],
                            rec[32 * g:32 * g + 1, 512 * c:512 * c + 512],
                            start=True, stop=True,
                            tile_position=(32 * g, 0),
                        )
                        bcs = sspool.tile([32, 512], F32, tag="bcs", name="bcs")
                        nc.scalar.copy(bcs[:], bc[:])
                        nc.vector.tensor_tensor(
                            att_sb[n][:, 512 * c:512 * c + 512],
                            araw[(n, c)][0:32, :], bcs[:], op=ALU.mult,
                        )
            for n in range(8):
                expt = []
                for j in range(8):
                    lt = bigps.tile([128, 1024], F32, tag="big", name="lt")
                    for c in range(2):
                        nc.tensor.matmul(
                            lt[:, 512 * c:512 * c + 512],
                            kp[n][:, 128 * j:128 * j + 128],
                            qp[n][:, 512 * c:512 * c + 512],
                            start=True, stop=True,
                        )
                    et = epool.tile([128, 1024], BF16, name="et")
                    nc.scalar.activation(et[:], lt[:], AF.Exp)
                    expt.append(et)
                aps2 = [abps.tile([33, 512], F32, tag="ab", name=f"aps{c}")
                        for c in range(2)]
                for j in range(8):
                    for c in range(2):
                        nc.tensor.matmul(
                            aps2[c][:],
                            vto[j][:, 33 * n:33 * n + 33],
                            expt[j][:, 512 * c:512 * c + 512],
                            start=(j == 0), stop=(j == 7),
                            skip_group_check=True,
                        )
                for c in range(2):
                    ar = sspool.tile([33, 512], F32, tag="araw",
                                     name="araw", bufs=16)
                    nc.vector.tensor_copy(ar[:], aps2[c][:])
                    araw[(n, c)] = ar
                    nc.sync.dma_start(sgt[(n // 4, c)][32 * (n % 4):32 * (n % 4) + 1, :],
                                      ar[32:33, :])
                if n == 3:
                    rec0 = group_recip(0)
                if n == 4:
                    group_norm(0, rec0)


            rec1 = group_recip(1)
            cps1 = cvps.tile([128, 1024], F32, tag="cv", name="cps1")
            conv_taps(1, cps1, 0, 5)
            group_norm(1, rec1)
            conv_taps(1, cps1, 5, 9)
            conv_finish(1, cps1)

            # ---- attn out-projection -> out rows 256..511 ----
            for o in range(2):
                ps = bigps.tile([128, 1024], F32, tag="big", name="pout")
                for n in range(8):
                    for c in range(2):
                        nc.tensor.matmul(
                            ps[:, 512 * c:512 * c + 512],
                            wo_sb[n][:, 128 * o:128 * o + 128],
                            att_sb[n][:, 512 * c:512 * c + 512],
                            start=(n == 0), stop=(n == 7),
                            skip_group_check=True,
                        )
                for c in range(2):
                    osb = opool.tile([128, 512], F32, name="osb")
                    nc.vector.tensor_copy(osb[:], ps[:, 512 * c:512 * c + 512])
                    nc.sync.dma_start(
                        out_d.ap()[256 + 128 * o:384 + 128 * o,
                                   512 * c:512 * c + 512], osb[:])


    nc.compile()
    return nc


_NC_CACHE = None


def kernel(x, w_qkv, w_conv, w_out, key_rel_h, key_rel_w):
    global _NC_CACHE, LAST_RESULT
    x = np.asarray(x, np.float32)
    w_qkv = np.asarray(w_qkv, np.float32)
    w_conv = np.asarray(w_conv, np.float32)
    w_out = np.asarray(w_out, np.float32)
    key_rel_h = np.asarray(key_rel_h, np.float32)
    key_rel_w = np.asarray(key_rel_w, np.float32)

    wq = w_qkv.copy()
    wq[:DK] *= DKH ** -0.5
    wqkt = _bf(wq[:1024].T)                      # (256, 1024)
    wvt = _bf(wq[1024:].T)                       # (256, 256)
    wconvt = _bf(w_conv.transpose(1, 2, 3, 0).reshape(256, 9 * 256))
    woutt = _bf(w_out.T)
    rel2 = np.concatenate([key_rel_w, key_rel_h], axis=1)  # (64, 126)
    relw = _bf(np.concatenate([rel2, rel2], axis=0))       # (128, 126)
    t = np.arange(L)
    ew = (t[None, :] // 32 == np.arange(32)[:, None]).astype(np.float32)
    eh = (t[None, :] % 32 == np.arange(32)[:, None]).astype(np.float32)
    e64 = np.concatenate([ew, eh], axis=0)
    econst = _bf(np.concatenate([e64, e64], axis=0))       # (128, 1024)

    shared = dict(wqkt=wqkt, wvt=wvt, wconvt=wconvt, woutt=woutt,
                  relw=relw, econst=econst)
    in_maps = []
    for b in range(B):
        xp = np.zeros((256, 34, 34), np.float32)
        xp[:, 1:33, 1:33] = x[b]
        in_maps.append(dict(shared, xpad=_bf(xp.reshape(256, 1156)),
                            xnat=_bf(x[b].reshape(256, 1024))))

    if _NC_CACHE is None:
        _NC_CACHE = build()
    res = run_bass_kernel_spmd(_NC_CACHE, in_maps, core_ids=list(range(8)),
                               trace=TRACE, **TRACE_KW)
    LAST_RESULT = res
    out = np.stack([res.results[i]["out"] for i in range(B)])
    return out.reshape(B, 512, H, W).astype(np.float32)
